# revision 7
# baseline (speedup 1.0000x reference)
"""PointerNet-style sparse attention kernel for Trainium2 (8 NeuronCores).

Reference computation (per full batch B=32):
    proj    = leaky_relu(src_encodings @ W_src.T, 0.01)        # (B,S,H*Q)
    weights = einsum('bshq,tbq->tbsh', proj.reshape(B,S,H,Q), query_vec)
    best    = argmax(query_vec @ W_opt.T, axis=-1)             # (T,B)
    w       = weights gathered at head 'best'                  # (T,B,S)
    out     = softmax(where(mask, -inf, w), axis=-1)           # (T,B,S)

Strategy: data-parallel over B across 8 cores (4 batches/core), weights
replicated.  Per local batch b:
  - PE-transpose src_b (S,E) -> srcT (E,S) in 128x128 blocks
  - projT[f,s] = Σ_e W_srcT[e,f]·srcT[e,s]  (fp32r matmuls, K=512 in 4 chunks)
    with fused leaky-relu on the PSUM->SBUF eviction (ACT Lrelu)
  - head-select without gather: one-hot(best) built from a tiny fp32 matmul
    (os = q_b @ W_optT) + row-max + is_ge; Qsel[t,(h,q)] = q_b[t,q]·onehot[t,h]
  - scores[t,s] = Σ_hq QselT[hq,t]·projT[hq,s]  (fp32r), with the token mask
    folded in as a rank-1 (ones ⊗ -1e30·mask) matmul appended to the same
    PSUM accumulation group
  - softmax over s: DVE row-max (negated) -> ACT Exp(bias=-max) with fused
    row-sum accum -> DVE reciprocal + scale -> DMA out
"""
import numpy as np

import concourse.bass as bass
import concourse.bacc as bacc
import concourse.mybir as mybir
import concourse.tile as tile
from concourse.masks import make_identity

F32 = mybir.dt.float32
F32R = mybir.dt.float32r
U8 = mybir.dt.uint8
AF = mybir.ActivationFunctionType
ALU = mybir.AluOpType
AX = mybir.AxisListType

N_CORES = 8
B, S, E = 32, 1024, 512
T, Q, H = 128, 256, 4
HQ = H * Q            # 1024 = total projection features
BL = B // N_CORES     # 4 local batches per core
EC = E // 128         # 4 K-chunks for matmul1
FC = HQ // 128        # 8 feature chunks (also K-chunks for matmul2)
NSPL = 2              # s splits of 512 (PSUM bank / fp32 moving max)

MASK_NEG = -1.0e30

# CoreSim does not implement Lrelu; tests flip this to AF.Relu for sim-only
# structural validation (HW semantics of Lrelu were verified separately).
LRELU_FUNC = AF.Lrelu


def build_nc() -> bass.Bass:
    nc = bacc.Bacc()
    src = nc.dram_tensor("src", [BL, S, E], F32, kind="ExternalInput")
    msk = nc.dram_tensor("msk", [BL, S], U8, kind="ExternalInput")
    qv = nc.dram_tensor("qv", [T, BL, Q], F32, kind="ExternalInput")
    wsrc = nc.dram_tensor("wsrc", [HQ, E], F32, kind="ExternalInput")
    wopt = nc.dram_tensor("wopt", [H, Q], F32, kind="ExternalInput")
    out = nc.dram_tensor("out", [T, BL, S], F32, kind="ExternalOutput")

    with tile.TileContext(nc) as tc:
        with (
            tc.tile_pool(name="const", bufs=1) as cp,
            tc.tile_pool(name="sb", bufs=2) as sb,
            tc.tile_pool(name="ps", bufs=6, space="PSUM") as ps,
        ):
            ident = cp.tile([128, 128], F32)
            make_identity(nc, ident[:])
            ones1f = cp.tile([1, 128], F32)
            nc.vector.memset(ones1f[:], 1.0)
            ones1 = cp.tile([1, 128], F32R)
            nc.vector.tensor_copy(ones1[:], ones1f[:])

            # ---- one-time: W_srcT  (E on partitions, F free), fp32r
            wsT = cp.tile([128, EC, HQ], F32R)
            wn = cp.tile([128, FC, E], F32)
            nc.sync.dma_start(wn[:], wsrc[:].rearrange("(c p) e -> p c e", p=128))
            for ec in range(EC):
                for half in range(2):
                    ptr = ps.tile([128, 512], F32, tag="ps")
                    for i in range(4):
                        fc = half * 4 + i
                        nc.tensor.transpose(
                            ptr[:, i * 128:(i + 1) * 128],
                            wn[:, fc, ec * 128:(ec + 1) * 128],
                            ident[:],
                        )
                    nc.vector.tensor_copy(
                        wsT[:, ec, half * 512:(half + 1) * 512], ptr[:])

            # ---- one-time: W_optT  (Q on partitions, H free), fp32
            woT = cp.tile([128, 2, H], F32)
            wo = cp.tile([H, Q], F32)
            nc.sync.dma_start(wo[:], wopt[:])
            pwo = ps.tile([128, 512], F32, tag="ps")
            for qc in range(2):
                nc.tensor.transpose(
                    pwo[:, qc * H:(qc + 1) * H],
                    wo[:, qc * 128:(qc + 1) * 128],
                    ident[0:H, 0:H],
                )
            nc.vector.tensor_copy(woT[:], pwo[:, 0:2 * H])

            for b in range(BL):
                # ---- load src_b and PE-transpose into srcT (f32r)
                sn = sb.tile([128, S // 128, E], F32, tag="sn")
                nc.sync.dma_start(
                    sn[:], src[b].rearrange("(c p) e -> p c e", p=128))
                st = sb.tile([128, EC, S], F32R, tag="st")
                for sc in range(S // 128):
                    ptr = ps.tile([128, 512], F32, tag="ps")
                    for ec in range(EC):
                        nc.tensor.transpose(
                            ptr[:, ec * 128:(ec + 1) * 128],
                            sn[:, sc, ec * 128:(ec + 1) * 128],
                            ident[:],
                        )
                    nc.vector.tensor_copy(
                        st[:, :, sc * 128:(sc + 1) * 128],
                        ptr[:].rearrange("p (c n) -> p c n", c=EC),
                    )

                # ---- matmul1: projT[f,s] += wsT.T @ srcT, fused leaky-relu
                pj = sb.tile([128, FC, S], F32R, tag="pj")
                for fc in range(FC):
                    for sp in range(NSPL):
                        pm = ps.tile([128, 512], F32, tag="ps")
                        for ec in range(EC):
                            nc.tensor.matmul(
                                pm[:],
                                wsT[:, ec, fc * 128:(fc + 1) * 128],
                                st[:, ec, sp * 512:(sp + 1) * 512],
                                start=(ec == 0), stop=(ec == EC - 1),
                            )
                        nc.scalar.activation(
                            pj[:, fc, sp * 512:(sp + 1) * 512], pm[:],
                            LRELU_FUNC, alpha=0.01)

                # ---- queries: load, transpose, head scores, one-hot, Qsel
                qn = sb.tile([128, Q], F32, tag="qn")
                nc.sync.dma_start(qn[:], qv[:, b, :])
                pq = ps.tile([128, 512], F32, tag="ps")
                for qc in range(2):
                    nc.tensor.transpose(
                        pq[:, qc * 128:(qc + 1) * 128],
                        qn[:, qc * 128:(qc + 1) * 128],
                        ident[:],
                    )
                qT = sb.tile([128, 2, 128], F32, tag="qT")
                nc.vector.tensor_copy(
                    qT[:], pq[:, 0:256].rearrange("p (c n) -> p c n", c=2))

                pos = ps.tile([128, 512], F32, tag="ps")
                for qc in range(2):
                    nc.tensor.matmul(
                        pos[:, 0:H], qT[:, qc, :], woT[:, qc, :],
                        start=(qc == 0), stop=(qc == 1),
                    )
                osmax = sb.tile([128, 1], F32, tag="osmax")
                nc.vector.tensor_reduce(osmax[:], pos[:, 0:H], AX.X, ALU.max)
                onehot = sb.tile([128, H], F32, tag="onehot")
                nc.vector.tensor_scalar(onehot[:], pos[:, 0:H], osmax[:], None,
                                        ALU.is_ge)

                qsel = sb.tile([128, H, Q], F32, tag="qsel")
                for h in range(H):
                    nc.vector.tensor_scalar(
                        qsel[:, h, :], qn[:], onehot[:, h:h + 1], None,
                        ALU.mult)

                qsT = sb.tile([128, FC, 128], F32R, tag="qsT")
                for j in range(2):
                    pqs = ps.tile([128, 512], F32, tag="ps")
                    for i in range(4):
                        c = j * 4 + i          # c = h*2 + qhalf
                        nc.tensor.transpose(
                            pqs[:, i * 128:(i + 1) * 128],
                            qsel[:, c // 2, (c % 2) * 128:(c % 2) * 128 + 128],
                            ident[:],
                        )
                    nc.vector.tensor_copy(
                        qsT[:, j * 4:(j + 1) * 4, :],
                        pqs[:].rearrange("p (c n) -> p c n", c=4))

                # ---- token mask -> additive row (-1e30 where masked)
                mu8 = sb.tile([1, S], U8, tag="mu8", bufs=1)
                nc.sync.dma_start(mu8[:], msk[b:b + 1, :])
                mf = sb.tile([1, S], F32, tag="mf", bufs=1)
                nc.vector.tensor_copy(mf[:], mu8[:])
                ma = sb.tile([1, S], F32R, tag="ma", bufs=1)
                nc.vector.tensor_scalar(ma[:], mf[:], MASK_NEG, None, ALU.mult)

                # ---- matmul2 + rank-1 mask add + softmax
                ex = sb.tile([128, S], F32, tag="ex")
                mx2 = sb.tile([128, NSPL], F32, tag="mx2")
                es2 = sb.tile([128, NSPL], F32, tag="es2")
                scps = []
                for sp in range(NSPL):
                    psc = ps.tile([128, 512], F32, tag="ps")
                    scps.append(psc)
                    for c in range(FC):
                        nc.tensor.matmul(
                            psc[:], qsT[:, c, :],
                            pj[:, c, sp * 512:(sp + 1) * 512],
                            start=(c == 0), stop=False,
                        )
                    nc.tensor.matmul(
                        psc[:], ones1[:], ma[:, sp * 512:(sp + 1) * 512],
                        start=False, stop=True,
                    )
                    nc.vector.tensor_reduce(
                        mx2[:, sp:sp + 1], psc[:], AX.X, ALU.max)

                negmax = sb.tile([128, 1], F32, tag="negmax")
                nc.vector.tensor_reduce(negmax[:], mx2[:], AX.X, ALU.max,
                                        negate=True)
                for sp in range(NSPL):
                    nc.scalar.activation(
                        ex[:, sp * 512:(sp + 1) * 512], scps[sp][:], AF.Exp,
                        bias=negmax[:], accum_out=es2[:, sp:sp + 1])

                essum = sb.tile([128, 1], F32, tag="essum")
                nc.vector.tensor_reduce(essum[:], es2[:], AX.X, ALU.add)
                rs = sb.tile([128, 1], F32, tag="rs")
                nc.vector.reciprocal(rs[:], essum[:])
                nc.vector.tensor_scalar(ex[:], ex[:], rs[:], None, ALU.mult)
                nc.sync.dma_start(out[:, b, :], ex[:])

    return nc


_NC_CACHE = None
LAST_RESULTS = None


def _get_nc():
    global _NC_CACHE
    if _NC_CACHE is None:
        nc = build_nc()
        nc.finalize()
        _NC_CACHE = nc
    return _NC_CACHE


def kernel(src_encodings, src_token_mask, query_vec, W_src, W_opt):
    from concourse.bass_utils import run_bass_kernel_spmd

    src_encodings = np.asarray(src_encodings, dtype=np.float32)
    src_token_mask = np.asarray(src_token_mask)
    query_vec = np.asarray(query_vec, dtype=np.float32)
    W_src = np.ascontiguousarray(np.asarray(W_src, dtype=np.float32))
    W_opt = np.ascontiguousarray(np.asarray(W_opt, dtype=np.float32))
    m8 = src_token_mask.astype(np.uint8)

    nc = _get_nc()
    in_maps = []
    for c in range(N_CORES):
        sl = slice(BL * c, BL * (c + 1))
        in_maps.append({
            "src": np.ascontiguousarray(src_encodings[sl]),
            "msk": np.ascontiguousarray(m8[sl]),
            "qv": np.ascontiguousarray(query_vec[:, sl, :]),
            "wsrc": W_src,
            "wopt": W_opt,
        })
    res = run_bass_kernel_spmd(nc, in_maps, core_ids=list(range(N_CORES)))
    global LAST_RESULTS
    LAST_RESULTS = res
    return np.concatenate([r["out"] for r in res.results], axis=1)


# revision 12
# speedup vs baseline: 10.4290x; 10.4290x over previous
"""PointerNet-style sparse attention kernel for Trainium2 (8 NeuronCores).

Reference computation (per full batch B=32):
    proj    = leaky_relu(src_encodings @ W_src.T, 0.01)        # (B,S,H*Q)
    weights = einsum('bshq,tbq->tbsh', proj.reshape(B,S,H,Q), query_vec)
    best    = argmax(query_vec @ W_opt.T, axis=-1)             # (T,B)
    w       = weights gathered at head 'best'                  # (T,B,S)
    out     = softmax(where(mask, -inf, w), axis=-1)           # (T,B,S)

Strategy: data-parallel over B across 8 cores (4 batches/core), weights
replicated.  Per local batch b:
  - PE-transpose src_b (S,E) -> srcT (E,S) in 128x128 blocks
  - projT[f,s] = Σ_e W_srcT[e,f]·srcT[e,s]  (fp32r matmuls, K=512 in 4 chunks)
    with fused leaky-relu on the PSUM->SBUF eviction (ACT Lrelu)
  - head-select without gather: one-hot(best) built from a tiny fp32 matmul
    (os = q_b @ W_optT) + row-max + is_ge; Qsel[t,(h,q)] = q_b[t,q]·onehot[t,h]
  - scores[t,s] = Σ_hq QselT[hq,t]·projT[hq,s]  (fp32r), with the token mask
    folded in as a rank-1 (ones ⊗ -1e30·mask) matmul appended to the same
    PSUM accumulation group
  - softmax over s: DVE row-max (negated) -> ACT Exp(bias=-max) with fused
    row-sum accum -> DVE reciprocal + scale -> DMA out
"""
import numpy as np

import concourse.bass as bass
import concourse.bacc as bacc
import concourse.mybir as mybir
import concourse.tile as tile
from concourse.masks import make_identity

F32 = mybir.dt.float32
F32R = mybir.dt.float32r
U8 = mybir.dt.uint8
AF = mybir.ActivationFunctionType
ALU = mybir.AluOpType
AX = mybir.AxisListType

N_CORES = 8
B, S, E = 32, 1024, 512
T, Q, H = 128, 256, 4
HQ = H * Q            # 1024 = total projection features
BL = B // N_CORES     # 4 local batches per core
EC = E // 128         # 4 K-chunks for matmul1
FC = HQ // 128        # 8 feature chunks (also K-chunks for matmul2)
NSPL = 2              # s splits of 512 (PSUM bank / fp32 moving max)

MASK_NEG = -1.0e30

# CoreSim does not implement Lrelu; tests flip this to AF.Relu for sim-only
# structural validation (HW semantics of Lrelu were verified separately).
LRELU_FUNC = AF.Lrelu


def build_nc(repeat: int = 1) -> bass.Bass:
    """repeat>1 replays the whole per-batch pipeline; used only for
    slope-based timing (dispatch overhead dwarfs one kernel execution)."""
    nc = bacc.Bacc()
    src = nc.dram_tensor("src", [BL, S, E], F32, kind="ExternalInput")
    msk = nc.dram_tensor("msk", [BL, S], U8, kind="ExternalInput")
    qv = nc.dram_tensor("qv", [T, BL, Q], F32, kind="ExternalInput")
    wsrc = nc.dram_tensor("wsrc", [HQ, E], F32, kind="ExternalInput")
    wopt = nc.dram_tensor("wopt", [H, Q], F32, kind="ExternalInput")
    out = nc.dram_tensor("out", [T, BL, S], F32, kind="ExternalOutput")
    ohd = nc.dram_tensor("ohd_scratch", [H, T], F32)

    with tile.TileContext(nc) as tc:
        with (
            tc.tile_pool(name="const", bufs=1) as cp,
            tc.tile_pool(name="sb", bufs=2) as sb,
            tc.tile_pool(name="ps", bufs=2, space="PSUM") as ps_tr,
            tc.tile_pool(name="psm", bufs=4, space="PSUM") as ps_mm,
            tc.tile_pool(name="pss", bufs=2, space="PSUM") as ps_sc,
        ):
            ident = cp.tile([128, 128], F32)
            make_identity(nc, ident[:])
            identr = cp.tile([128, 128], F32R)
            nc.vector.tensor_copy(identr[:], ident[:])
            ones1f = cp.tile([1, 128], F32)
            nc.vector.memset(ones1f[:], 1.0)
            ones1 = cp.tile([1, 128], F32R)
            nc.vector.tensor_copy(ones1[:], ones1f[:])

            # ---- one-time: W_srcT  (E on partitions, F free), fp32r
            wsT = cp.tile([128, EC, HQ], F32R)
            wn = cp.tile([128, FC, E], F32)
            nc.sync.dma_start(wn[:], wsrc[:].rearrange("(c p) e -> p c e", p=128))
            for ec in range(EC):
                for half in range(2):
                    ptr = ps_tr.tile([128, 512], F32, tag="ps")
                    for i in range(4):
                        fc = half * 4 + i
                        nc.tensor.transpose(
                            ptr[:, i * 128:(i + 1) * 128],
                            wn[:, fc, ec * 128:(ec + 1) * 128],
                            ident[:],
                        )
                    nc.vector.tensor_copy(
                        wsT[:, ec, half * 512:(half + 1) * 512], ptr[:])

            # ---- one-time: W_optT  (Q on partitions, H free), fp32
            woT = cp.tile([128, 2, H], F32)
            wo = cp.tile([H, Q], F32)
            nc.sync.dma_start(wo[:], wopt[:])
            pwo = ps_tr.tile([128, 512], F32, tag="ps")
            for qc in range(2):
                nc.tensor.transpose(
                    pwo[:, qc * H:(qc + 1) * H],
                    wo[:, qc * 128:(qc + 1) * 128],
                    ident[0:H, 0:H],
                )
            nc.vector.tensor_copy(woT[:], pwo[:, 0:2 * H])

            for rep in range(repeat):
              for b in range(BL):
                # ---- load src_b and PE-transpose into srcT (f32r)
                sn = sb.tile([128, S // 128, E], F32R, tag="sn")
                nc.sync.dma_start(
                    sn[:], src[b].bitcast(F32R).rearrange("(c p) e -> p c e", p=128))
                st = sb.tile([128, EC, S], F32R, tag="st")
                for sc in range(S // 128):
                    ptr = ps_tr.tile([128, 512], F32R, tag="ps")
                    for ec in range(EC):
                        nc.tensor.transpose(
                            ptr[:, ec * 128:(ec + 1) * 128],
                            sn[:, sc, ec * 128:(ec + 1) * 128],
                            identr[:],
                        )
                    nc.vector.tensor_copy(
                        st[:, :, sc * 128:(sc + 1) * 128],
                        ptr[:].rearrange("p (c n) -> p c n", c=EC),
                    )

                # ---- matmul1: projT[f,s] += wsT.T @ srcT, fused leaky-relu
                pj = sb.tile([128, FC, S], F32R, tag="pj")
                for fc in range(FC):
                    for sp in range(NSPL):
                        pm = ps_mm.tile([128, 512], F32, tag="ps")
                        for ec in range(EC):
                            nc.tensor.matmul(
                                pm[:],
                                wsT[:, ec, fc * 128:(fc + 1) * 128],
                                st[:, ec, sp * 512:(sp + 1) * 512],
                                start=(ec == 0), stop=(ec == EC - 1),
                            )
                        nc.scalar.activation(
                            pj[:, fc, sp * 512:(sp + 1) * 512], pm[:],
                            LRELU_FUNC, alpha=0.01)

                # ---- queries: load, transpose, head scores, one-hot, Qsel
                qn = sb.tile([128, Q], F32, tag="qn")
                nc.sync.dma_start(qn[:], qv[:, b, :])
                pq = ps_tr.tile([128, 512], F32, tag="ps")
                for qc in range(2):
                    nc.tensor.transpose(
                        pq[:, qc * 128:(qc + 1) * 128],
                        qn[:, qc * 128:(qc + 1) * 128],
                        ident[:],
                    )
                qT = sb.tile([128, 2, 128], F32, tag="qT")
                nc.vector.tensor_copy(
                    qT[:], pq[:, 0:256].rearrange("p (c n) -> p c n", c=2))

                pos = ps_tr.tile([128, 512], F32, tag="ps")
                for qc in range(2):
                    nc.tensor.matmul(
                        pos[:, 0:H], qT[:, qc, :], woT[:, qc, :],
                        start=(qc == 0), stop=(qc == 1),
                    )
                osmax = sb.tile([128, 1], F32, tag="osmax")
                nc.vector.tensor_reduce(osmax[:], pos[:, 0:H], AX.X, ALU.max)
                onehot = sb.tile([128, H], F32, tag="onehot")
                nc.vector.tensor_scalar(onehot[:], pos[:, 0:H], osmax[:], None,
                                        ALU.is_ge)

                # one-hot -> [H, T] via a tiny PE transpose, bounce through
                # DRAM to broadcast across partitions, then mask the
                # transposed queries on DVE: qsT[(h,qq), t] = qT[qq,t]*oh[h,t]
                poh = ps_tr.tile([128, 512], F32, tag="ps")
                nc.tensor.transpose(poh[0:H, 0:T], onehot[:], ident[:])
                ohT = sb.tile([H, T], F32, tag="ohT", bufs=2)
                nc.vector.tensor_copy(ohT[:], poh[0:H, 0:T])
                nc.sync.dma_start(ohd[:], ohT[:])
                ohB = sb.tile([128, H, T], F32, tag="ohB", bufs=2)
                nc.sync.dma_start(ohB[:], ohd[:].to_broadcast((128, H, T)))

                qsT = sb.tile([128, FC, 128], F32R, tag="qsT")
                for c in range(FC):
                    nc.vector.tensor_tensor(
                        qsT[:, c, :], qT[:, c % 2, :], ohB[:, c // 2, :],
                        ALU.mult)

                # ---- token mask -> additive row (-1e30 where masked)
                mu8 = sb.tile([1, S], U8, tag="mu8", bufs=1)
                nc.sync.dma_start(mu8[:], msk[b:b + 1, :])
                mf = sb.tile([1, S], F32, tag="mf", bufs=1)
                nc.vector.tensor_copy(mf[:], mu8[:])
                ma = sb.tile([1, S], F32R, tag="ma", bufs=1)
                nc.vector.tensor_scalar(ma[:], mf[:], MASK_NEG, None, ALU.mult)

                # ---- matmul2 + rank-1 mask add + softmax
                ex = sb.tile([128, S], F32, tag="ex")
                mx2 = sb.tile([128, NSPL], F32, tag="mx2")
                es2 = sb.tile([128, NSPL], F32, tag="es2")
                scps = []
                for sp in range(NSPL):
                    psc = ps_sc.tile([128, 512], F32, tag="ps")
                    scps.append(psc)
                    for c in range(FC):
                        nc.tensor.matmul(
                            psc[:], qsT[:, c, :],
                            pj[:, c, sp * 512:(sp + 1) * 512],
                            start=(c == 0), stop=False,
                        )
                    nc.tensor.matmul(
                        psc[:], ones1[:], ma[:, sp * 512:(sp + 1) * 512],
                        start=False, stop=True,
                    )
                    nc.vector.tensor_reduce(
                        mx2[:, sp:sp + 1], psc[:], AX.X, ALU.max)

                negmax = sb.tile([128, 1], F32, tag="negmax")
                nc.vector.tensor_reduce(negmax[:], mx2[:], AX.X, ALU.max,
                                        negate=True)
                for sp in range(NSPL):
                    nc.scalar.activation(
                        ex[:, sp * 512:(sp + 1) * 512], scps[sp][:], AF.Exp,
                        bias=negmax[:], accum_out=es2[:, sp:sp + 1])

                essum = sb.tile([128, 1], F32, tag="essum")
                nc.vector.tensor_reduce(essum[:], es2[:], AX.X, ALU.add)
                rs = sb.tile([128, 1], F32, tag="rs")
                nc.vector.reciprocal(rs[:], essum[:])
                nc.vector.tensor_scalar(ex[:], ex[:], rs[:], None, ALU.mult)
                nc.sync.dma_start(out[:, b, :], ex[:])

    return nc


_NC_CACHE = None
LAST_RESULTS = None


def _get_nc():
    global _NC_CACHE
    if _NC_CACHE is None:
        nc = build_nc()
        nc.finalize()
        _NC_CACHE = nc
    return _NC_CACHE


def kernel(src_encodings, src_token_mask, query_vec, W_src, W_opt):
    from concourse.bass_utils import run_bass_kernel_spmd

    src_encodings = np.asarray(src_encodings, dtype=np.float32)
    src_token_mask = np.asarray(src_token_mask)
    query_vec = np.asarray(query_vec, dtype=np.float32)
    W_src = np.ascontiguousarray(np.asarray(W_src, dtype=np.float32))
    W_opt = np.ascontiguousarray(np.asarray(W_opt, dtype=np.float32))
    m8 = src_token_mask.astype(np.uint8)

    nc = _get_nc()
    in_maps = []
    for c in range(N_CORES):
        sl = slice(BL * c, BL * (c + 1))
        in_maps.append({
            "src": np.ascontiguousarray(src_encodings[sl]),
            "msk": np.ascontiguousarray(m8[sl]),
            "qv": np.ascontiguousarray(query_vec[:, sl, :]),
            "wsrc": W_src,
            "wopt": W_opt,
        })
    res = run_bass_kernel_spmd(nc, in_maps, core_ids=list(range(N_CORES)))
    global LAST_RESULTS
    LAST_RESULTS = res
    return np.concatenate([r["out"] for r in res.results], axis=1)
